# revision 17
# baseline (speedup 1.0000x reference)
"""Bass/Trainium2 kernel for nn_Attend (masked+biased multi-head attention).

Problem (hardcoded): b=2, n=2048, d_model=512, h=8 heads, d=64.
  out[b,h,i,:] = softmax_j(q_h[b,i]·k_h[b,j]*scale masked + bias[h,i,j]) @ v_h[b]

Sharding: head-parallel across the 8 NeuronCores (core c <-> head c), both
batches on every core.  No cross-core communication.

Per-core device algorithm (everything stored "transposed", j on partitions):
  S_T[j,i]   = sum_d kT[d,j] * qT_scaled[d,i]          (PE, K=64)
  S_T       += bias_T[j,i]                             (PE identity-matmul accumulate)
  S_T       += -1e9 * mask_T[j,i]                      (DVE scalar_tensor_tensor)
  E_T        = exp(S_T)                                (ACT, PSUM->SBUF)
  outT[d,i], Z[i] = sum_j v_aug[j,d-or-ones] * E_T[j,i] (PE, accumulate over j)
  out[i,d]   = transpose(outT)[i,d] / Z[i]             (PE transpose + DVE)

The j-loop streams bias_T/mask_T tiles (the dominant HBM traffic) through
SBUF once; this problem is HBM-bandwidth bound on the bias matrix.
"""

import os
from contextlib import ExitStack

import numpy as np

B = 2
N = 2048
DM = 512
H = 8
D = 64  # head dim

JB = 128          # j rows per block (partition dim)
NJ = N // JB      # 16 j blocks
IC = 512          # i columns per matmul (one PSUM bank of fp32)
IH = 1024         # i columns per exp op (2 banks)
NEG = -1.0e9
BIG = 4096.0

# --- tunables ---------------------------------------------------------------
CFG = {
    "e_dtype": os.environ.get("ATT_E_DTYPE", "bf16"),      # f32 | bf16
    "v_dtype": os.environ.get("ATT_V_DTYPE", "bf16"),      # f32 | bf16
    "mask_mode": os.environ.get("ATT_MASK_MODE", "stt"),  # stt | postmul
    "mm_dtype": os.environ.get("ATT_MM_DTYPE", "bf16"),   # f32 | f32r | bf16
    "bias_on_pe": os.environ.get("ATT_BIAS_ON_PE", "1") == "1",
    "s_bufs": int(os.environ.get("ATT_S_BUFS", "4")),
    "gps_frac8": int(os.environ.get("ATT_GPS_FRAC8", "3")),
    "in_bufs": int(os.environ.get("ATT_IN_BUFS", "3")),
}


def _dt(mybir, name):
    return {"f32": mybir.dt.float32, "bf16": mybir.dt.bfloat16}[name]


def build_program(scale: float, cfg=None):
    """Build the single-core SPMD Bass program (same NEFF on all 8 cores)."""
    import concourse.bass as bass
    import concourse.tile as tile
    from concourse import bacc, mybir

    cfg = dict(CFG, **(cfg or {}))
    e_dt = _dt(mybir, cfg["e_dtype"])
    v_dt = _dt(mybir, cfg["v_dtype"])
    f32 = mybir.dt.float32
    u8 = mybir.dt.uint8
    Exp = mybir.ActivationFunctionType.Exp
    Copy = mybir.ActivationFunctionType.Copy

    nc = bacc.Bacc()
    mdt = {"f32r": mybir.dt.float32r, "bf16": mybir.dt.bfloat16,
           "f32": f32}[cfg["mm_dtype"]]

    qT = nc.declare_dram_parameter("qT", [B, D, N], mdt, isOutput=False)
    kT = nc.declare_dram_parameter("kT", [B, D, N], mdt, isOutput=False)
    vh = nc.declare_dram_parameter("v", [B, N, D], v_dt, isOutput=False)
    biasT = nc.declare_dram_parameter("biasT", [N, N], mdt, isOutput=False)
    maskT = nc.declare_dram_parameter("maskT", [B, N, N], u8, isOutput=False)
    ident = nc.declare_dram_parameter("ident", [128, 128], f32, isOutput=False)
    # identity scaled by 1/scale: the PE bias-add injects bias/scale, and the
    # exp applies the scale to the whole pre-activation sum
    idents = nc.declare_dram_parameter("idents", [128, 128], mdt, isOutput=False)
    out = nc.declare_dram_parameter("out", [B, N, D], f32, isOutput=True)

    with ExitStack() as ctx:
        tc = ctx.enter_context(tile.TileContext(nc))
        singles = ctx.enter_context(tc.tile_pool(name="singles", bufs=1))
        ins = ctx.enter_context(tc.tile_pool(name="ins", bufs=cfg["in_bufs"]))
        vs = ctx.enter_context(tc.tile_pool(name="vs", bufs=3))
        es = ctx.enter_context(tc.tile_pool(name="es", bufs=3))
        drains = ctx.enter_context(tc.tile_pool(name="drains", bufs=2))
        smalls = ctx.enter_context(tc.tile_pool(name="smalls", bufs=8))
        spool = ctx.enter_context(tc.tile_pool(name="spool", bufs=cfg["s_bufs"], space="PSUM"))
        opool = ctx.enter_context(tc.tile_pool(name="opool", bufs=1, space="PSUM"))

        # ---- one-time loads -------------------------------------------------
        ident_sb = singles.tile([128, 128], f32, tag="ident")
        nc.sync.dma_start(out=ident_sb, in_=ident[:, :])
        idents_sb = singles.tile([128, 128], mdt, tag="idents")
        nc.sync.dma_start(out=idents_sb, in_=idents[:, :])
        negbig = singles.tile([128, 1], f32, tag="negbig")
        nc.vector.memset(negbig, -BIG)

        # q/k stored K-padded to 128 rows (zeros below row 64): full-K
        # matmuls keep the PE activity monitor warm at no stream cost
        qT_sb, kT_sb = [], []
        for b in range(B):
            qb = singles.tile([128, N], mdt, name=f"qTs{b}", tag=f"qT{b}")
            nc.sync.dma_start(out=qb[0:D, :], in_=qT[b])
            nc.vector.memset(qb[D:128, :], 0.0)
            qT_sb.append(qb)
            kb = singles.tile([128, N], mdt, name=f"kTs{b}", tag=f"kT{b}")
            nc.sync.dma_start(out=kb[0:D, :], in_=kT[b])
            nc.vector.memset(kb[D:128, :], 0.0)
            kT_sb.append(kb)

        # ---- main loop ------------------------------------------------------
        for b in range(B):
            # PV accumulators for the 4 i-chunks, live across the whole j loop
            pv = [opool.tile([D + 1, IC], f32, name=f"pv{ic}", tag=f"pv{ic}") for ic in range(4)]

            for j in range(NJ):
                bias_sb = ins.tile([JB, N], mdt, tag="bias")
                nc.sync.dma_start(out=bias_sb, in_=biasT[j * JB:(j + 1) * JB, :])
                mask_sb = ins.tile([JB, N], e_dt, tag="mask")
                # SWDGE cast-DMA: u8 {0,1} in HBM -> bf16 {0.0,1.0} in SBUF
                nc.gpsimd.dma_start(out=mask_sb, in_=maskT[b, j * JB:(j + 1) * JB, :])

                v_aug = vs.tile([JB, D + 1], v_dt, tag="vaug")
                nc.sync.dma_start(out=v_aug[:, 0:D], in_=vh[b, j * JB:(j + 1) * JB, :])
                nc.vector.memset(v_aug[:, D:D + 1], 1.0)

                e_sb = es.tile([JB, N], e_dt, tag="e")

                for c in range(N // IC):
                    csl = bass.ts(c, IC)
                    s_ps = spool.tile([JB, IC], f32, tag="s")
                    nc.tensor.matmul(
                        s_ps,
                        lhsT=kT_sb[b][:, j * JB:(j + 1) * JB],
                        rhs=qT_sb[b][:, csl],
                        start=True, stop=not cfg["bias_on_pe"],
                    )
                    if cfg["bias_on_pe"]:
                        nc.tensor.matmul(
                            s_ps,
                            lhsT=idents_sb,
                            rhs=bias_sb[:, csl],
                            start=False, stop=True,
                        )
                    else:
                        nc.vector.scalar_tensor_tensor(
                            out=s_ps, in0=bias_sb[:, csl], scalar=1.0,
                            in1=s_ps, op0=mybir.AluOpType.mult,
                            op1=mybir.AluOpType.add,
                        )
                    # maskT holds the INVERTED mask (1.0 = keep), bf16
                    nc.scalar.activation(out=e_sb[:, csl], in_=s_ps, func=Exp,
                                         scale=float(scale))
                    # zero masked entries: bf16 2x-mode multiply on DVE
                    nc.vector.tensor_tensor(
                        out=e_sb[:, csl], in0=e_sb[:, csl],
                        in1=mask_sb[:, csl], op=mybir.AluOpType.mult,
                    )
                    nc.tensor.matmul(
                        pv[c],
                        lhsT=v_aug,
                        rhs=e_sb[:, csl],
                        start=(j == 0), stop=(j == NJ - 1),
                    )

            # ---- drain batch b: normalize + transpose -----------------------
            ot_sb = drains.tile([D + 1, N], f32, tag="ot")
            for ic in range(4):
                nc.scalar.activation(out=ot_sb[:, bass.ts(ic, IC)], in_=pv[ic], func=Copy)

            ostage = drains.tile([128, N // 128 * D], f32, tag="ostage")
            for t in range(N // 128):
                t_ps = spool.tile([128, D + 1], f32, tag="s")
                nc.tensor.transpose(
                    t_ps, ot_sb[:, t * 128:(t + 1) * 128], ident_sb[0:D + 1, 0:D + 1],
                )
                rz = smalls.tile([128, 1], f32, tag="rz")
                nc.vector.reciprocal(rz, t_ps[:, D:D + 1])
                nc.vector.tensor_scalar_mul(ostage[:, bass.ts(t, D)], t_ps[:, 0:D], rz)

            nc.sync.dma_start(
                out=out[b].rearrange("(t p) d -> p t d", p=128),
                in_=ostage.rearrange("p (t d) -> p t d", d=D),
            )

    nc.compile()
    return nc


_PROG_CACHE = {}


def _get_program(scale: float):
    key = (round(float(scale), 9), tuple(sorted(CFG.items())))
    if key not in _PROG_CACHE:
        _PROG_CACHE[key] = build_program(float(scale))
    return _PROG_CACHE[key]


_SCALE_HOLDER = [0.125]


def make_in_maps(q, k, v, mask, bias):
    import ml_dtypes
    mm_np = {"f32": np.float32, "f32r": np.float32,
             "bf16": ml_dtypes.bfloat16}[CFG["mm_dtype"]]
    v_np = {"f32": np.float32, "bf16": ml_dtypes.bfloat16}[CFG["v_dtype"]]
    q = np.asarray(q, dtype=np.float32)
    k = np.asarray(k, dtype=np.float32)
    v = np.asarray(v, dtype=np.float32)
    mask_u8 = np.asarray(mask).astype(np.uint8)  # (B,1,N,N), True==masked
    bias = np.asarray(bias, dtype=np.float32)    # (1,H,N,N)
    eye = np.eye(128, dtype=np.float32)
    scale_f = float(np.asarray(_SCALE_HOLDER[0]))
    eyes = (eye / scale_f).astype(mm_np)
    eye_mm = eye.astype(mm_np)

    in_maps = []
    for h in range(H):
        sl = slice(h * D, (h + 1) * D)
        in_maps.append({
            "qT": np.ascontiguousarray(q[:, :, sl].transpose(0, 2, 1)).astype(mm_np),
            "kT": np.ascontiguousarray(k[:, :, sl].transpose(0, 2, 1)).astype(mm_np),
            "v": np.ascontiguousarray(v[:, :, sl]).astype(v_np),
            "biasT": np.ascontiguousarray(bias[0, h].T).astype(mm_np),
            "maskT": np.ascontiguousarray((1 - mask_u8)[:, 0].transpose(0, 2, 1)),
            "ident": eye,
            "idents": eyes,
        })
    return in_maps


def run(q, k, v, scale, mask, bias, trace=False, trace_kwargs=None):
    from concourse.bass_utils import run_bass_kernel_spmd

    _SCALE_HOLDER[0] = float(np.asarray(scale))
    nc = _get_program(float(np.asarray(scale)))
    in_maps = make_in_maps(q, k, v, mask, bias)
    res = run_bass_kernel_spmd(
        nc, in_maps, core_ids=list(range(H)),
        trace=trace, **(trace_kwargs or {}),
    )
    outs = [np.asarray(res.results[h]["out"]) for h in range(H)]
    full = np.stack(outs, axis=1).astype(np.float32)  # (B, H, N, D)
    return full, res


def kernel(q, k, v, scale, mask, bias):
    full, _ = run(q, k, v, scale, mask, bias, trace=False)
    return full
